# revision 11
# baseline (speedup 1.0000x reference)
"""Additive attention (B=4, Q=512, K=512, D=256, H=256) on 8 TRN2 NeuronCores.

Sharding: data-parallel over query rows. Core c owns q-rows [c*64, (c+1)*64)
of every batch; every core holds all keys/values/weights. No collectives.

Per-core pipeline (per batch b, JIT-specialized to valid_lens[b]):
  PE : qf = Wq^T q, kf = Wk^T k (all batches upfront; bf16, H on partitions)
  then each q-row takes one of two tanh paths, balanced across engines:
   ACT path: pre[h,k] = kf[h,k] + qf[h,q] (DVE or Pool tensor_scalar),
             feat = tanh(pre) on ACT (batched groups of 16 q's)
   DVE path: xc = clamp(kf + qf, +-3.0) (two tensor_scalar), then a single
             8-stage custom-DVE op TANH7_ANT evaluates an odd deg-7 minimax
             polynomial: feat = xc*(c0 + c1 t + c2 t^2 + c3 t^3), t = xc^2
  PE : scoresT[k,q] = (feat as weights)^T wv  (k on PSUM partitions)
  ACT: expT[k,q] = exp(scoresT)  (masked tail rows biased to -40)
  PE : out[q,:] = expT^T @ [values | 1]  (ones col gives the softmax denom)
  DVE: out[:, :D] *= 1/out[:, D]
"""

from contextlib import ExitStack

import ml_dtypes
import numpy as np

import concourse.bass as bass
import concourse.mybir as mybir
import concourse.tile as tile
from concourse import bacc
from concourse.bass_utils import run_bass_kernel_spmd

B, Q, K, D, H = 4, 512, 512, 256, 256
DA = D + 1  # values with an appended ones column
NCORES = 8
QL = Q // NCORES  # 64 q-rows per core
GS = 16  # q's per ACT tanh group
PCH = 6  # q's per DVE poly chunk
BF16 = mybir.dt.bfloat16
F16 = mybir.dt.float16
F32 = mybir.dt.float32
AF = mybir.ActivationFunctionType
OP = mybir.AluOpType

# Custom-DVE tanh: odd deg-7 minimax on [-XC, XC], one 8-stage instruction
# (tanh(x) ~ x*(c0 + c1 t + c2 t^2 + c3 t^3), t = x^2; |err| < 2.2e-2 + clamp
# err 5e-3 -- end-to-end rel err ~7e-3 with half the q-rows on this path).
XC = 3.0
T7 = [0.95467697, -0.21116664, 0.02812073, -0.00138176]  # c0..c3

_build_cache: dict = {}
last_results = None  # BassKernelResults of the most recent kernel() call


def _register_tanh7():
    import concourse.dve_ops as dops
    from concourse.dve_ops import DveOp
    from concourse.dve_spec import Spec, Src0, Src1, C0, C1, C2, sq, lower
    from concourse.dve_uop import DveOpSpec

    if "TANH7_ANT" in dops._SUB_OPCODE_FOR_NAME:
        return next(op for op in dops.OPS if op.name == "TANH7_ANT")

    def _tanh7_ref(in0, in1, c0, c1, c2):
        x = in0.astype(np.float32).reshape(in0.shape[0], -1)
        if isinstance(in1, np.ndarray):
            in1 = np.asarray(in1, np.float32).reshape(in1.shape[0], -1)
            if in1.shape[1] not in (1, x.shape[1]):
                in1 = in1[:, :1]
        t = x * x
        u = in1 * t + c0
        u = u * t + c1
        u = u * t + c2
        return u * x

    _t = sq(Src0)
    _u = Src1 * _t + C0
    _u = _u * _t + C1
    _u = _u * _t + C2
    op = DveOp("TANH7_ANT", Spec(body=_u * Src0, reference=_tanh7_ref),
               subdim=False, uops_sha={})
    row = max(dops._SUB_OPCODE_FOR_NAME.values()) + 1
    dops.OPS.append(op)
    dops._SUB_OPCODE_FOR_NAME[op.name] = row
    dops.CUSTOM_DVE_SPECS[op.name] = op.spec
    for ver in ("v3",):
        s = DveOpSpec(name=op.name, opcode=row, uops=lower(op.spec, ver=ver),
                      rd1_en=True)
        op.uops_sha[ver] = s.sha(ver)
    return op


TANH7 = _register_tanh7()


def _k_use(v: int) -> int:
    # columns actually computed for a batch: valid len rounded up to a
    # multiple of 4 (even FD keeps DVE tensor_scalar in fast mode)
    return min(K, max(4, ((int(v) + 3) // 4) * 4))


def _plan(ku):
    """Per-batch (n_poly, n_pool): q-rows on the DVE-poly path and q-rows
    whose ACT-path preadd runs on Pool/gpsimd. Greedy 2-D grid on the
    analytic per-engine cost model (ns)."""
    def tot(fp, fl):
        act = dve = pool = 0.0
        for Ku in ku:
            np_ = int(round(fp * QL))
            nl_ = int(round(fl * QL))
            if np_ + nl_ > QL:
                nl_ = QL - np_
            na = QL - np_
            nd = max(na - nl_, 0)
            act += na * 2 * Ku * 0.8333
            dve += nd * 2 * (0.2604 * Ku + 60) \
                + np_ * 2 * (0.2604 * Ku + 60) \
                + np_ * 2 * Ku * (0.2604 + 1.0417) + 120
            pool += nl_ * 2 * (1.3889 * Ku + 156)
        return max(act, dve, pool)

    best = (1e18, 0.0, 0.0)
    for fp10 in range(0, 40):
        for fl10 in range(0, 55):
            v = tot(fp10 / 64.0, fl10 / 64.0)
            if v < best[0]:
                best = (v, fp10 / 64.0, fl10 / 64.0)
    _, fp, fl = best
    out = []
    for Ku in ku:
        np_ = int(round(fp * QL))
        nl_ = min(int(round(fl * QL)), QL - np_)
        out.append((np_, nl_))
    return out


def build(valid_lens, repeat: int = 1) -> bacc.Bacc:
    valid = tuple(int(v) for v in valid_lens)
    ku = [_k_use(v) for v in valid]

    nc = bacc.Bacc("TRN2", target_bir_lowering=False, debug=False,
                   enable_asserts=False)

    qT = nc.dram_tensor("qT", [128, 2, B * QL], BF16, kind="ExternalInput").ap()
    kT = nc.dram_tensor("kT", [B, 128, 2, K], BF16, kind="ExternalInput").ap()
    vals = nc.dram_tensor("vals", [B, 128, 4, DA], BF16, kind="ExternalInput").ap()
    wq = nc.dram_tensor("wq", [128, 2, H], BF16, kind="ExternalInput").ap()
    wk = nc.dram_tensor("wk", [128, 2, H], BF16, kind="ExternalInput").ap()
    wv2 = nc.dram_tensor("wv2", [128, 2], BF16, kind="ExternalInput").ap()
    wv16 = nc.dram_tensor("wv16", [128, 2], F16, kind="ExternalInput").ap()
    # per-batch mask bias column for the last k-tile's exp: 0 on valid rows,
    # -40 on the rounded-up tail (exp(-40) ~ 4e-18 ~ 0)
    mb = nc.dram_tensor("mb", [128, B], F32, kind="ExternalInput").ap()
    # constant c3 plane for the custom op's second stream ([P,1] broadcast
    # src1 crashes this firmware; a real full-size tensor works)
    c3rep = nc.dram_tensor("c3rep", [128, PCH * 2 * K], F16,
                           kind="ExternalInput").ap()
    out = nc.dram_tensor("out", [B, QL, D], F32, kind="ExternalOutput").ap()

    plan = _plan(ku)
    mku = max(ku)

    with tile.TileContext(nc) as tc, ExitStack() as ctx:
        cp = ctx.enter_context(tc.tile_pool(name="consts", bufs=1))
        sb = ctx.enter_context(tc.tile_pool(name="sbuf", bufs=2))
        exp_pool = ctx.enter_context(tc.tile_pool(name="expp", bufs=5))
        small = ctx.enter_context(tc.tile_pool(name="small", bufs=4))
        pre_pool = ctx.enter_context(tc.tile_pool(name="pre", bufs=2))
        pred_pool = ctx.enter_context(tc.tile_pool(name="pred", bufs=2))
        res_pool = ctx.enter_context(tc.tile_pool(name="res", bufs=2))
        ps_proj = ctx.enter_context(tc.tile_pool(name="ps_proj", bufs=4, space="PSUM"))
        ps_sc = ctx.enter_context(tc.tile_pool(name="ps_sc", bufs=2, space="PSUM"))
        ps_out = ctx.enter_context(tc.tile_pool(name="ps_out", bufs=2, space="PSUM"))

        # prime the ACT table load (tanh/exp share one set) at t~0 so the
        # ~2.7us load is off the critical path
        primer = cp.tile([1, 1], F32, tag="primer")
        nc.gpsimd.memset(primer[:, :], 0.0)
        nc.scalar.activation(primer[:, :], primer[:, :], AF.Tanh)

        wq_sb = cp.tile([128, 2, H], BF16, tag="wq")
        wk_sb = cp.tile([128, 2, H], BF16, tag="wk")
        wv_sb = cp.tile([128, 2], BF16, tag="wv")
        wv16_sb = cp.tile([128, 2], F16, tag="wv16")
        qT_sb = cp.tile([128, 2, B * QL], BF16, tag="qT")
        mb_sb = cp.tile([128, B], F32, tag="mb")
        c3_sb = cp.tile([128, PCH * 2 * K], F16, tag="c3")

        # process batches small-to-large so the serial prologue (DMA ->
        # projection -> preadd -> first tanh) is as short as possible
        border = sorted(range(B), key=lambda b: ku[b])
        seq = [b for _ in range(repeat) for b in border]
        b0 = seq[0]

        # spread the prologue-critical DMAs over different engines' DMA
        # queues so they transfer in parallel
        kT_sb = cp.tile([128, B, 2, K], BF16, tag="kT")
        nc.scalar.dma_start(kT_sb[:, b0], kT[b0])
        nc.sync.dma_start(wk_sb[:, :, :], wk[:, :, :])
        nc.sync.dma_start(wq_sb[:, :, :], wq[:, :, :])
        nc.gpsimd.dma_start(qT_sb[:, :, :], qT[:, :, :])
        nc.sync.dma_start(wv_sb[:, :], wv2[:, :])
        nc.sync.dma_start(wv16_sb[:, :], wv16[:, :])
        nc.sync.dma_start(mb_sb[:, :], mb[:, :])
        nc.scalar.dma_start(c3_sb[:, :], c3rep[:, :])
        for b in seq[1:]:
            nc.sync.dma_start(kT_sb[:, b], kT[b])

        # kf projections for every batch upfront (PE is far from saturated);
        # kf[h, k] bf16, H split in two 128-halves
        kf_tiles = {}

        def project_kf(b, uniq):
            Ku_ = ku[b]
            kf_t = cp.tile([128, 2, K], BF16, tag=f"kf{b}", name=f"kf_{uniq}")
            for h2 in range(2):
                ps = ps_proj.tile([128, K], F32, tag="proj",
                                  name=f"kfp_{uniq}_{h2}")
                for dt in range(2):
                    nc.tensor.matmul(
                        ps[:, :Ku_],
                        lhsT=wk_sb[:, dt, bass.ts(h2, 128)],
                        rhs=kT_sb[:, b, dt, :Ku_],
                        start=(dt == 0), stop=(dt == 1),
                    )
                nc.vector.tensor_copy(kf_t[:, h2, :Ku_], ps[:, :Ku_])
            return kf_t

        kf_tiles[b0] = project_kf(b0, "p0")

        # qf[h, (b,q)] for all batches, f32 (tensor_scalar's per-partition
        # scalar operand must be float32; copies go on the ACT engine)
        qf_sb = cp.tile([128, 2, B * QL], F32, tag="qf")
        for h2 in range(2):
            ps = ps_proj.tile([128, B * QL], F32, tag="proj",
                              name=f"qfp_{h2}")
            for dt in range(2):
                nc.tensor.matmul(
                    ps[:, :],
                    lhsT=wq_sb[:, dt, bass.ts(h2, 128)],
                    rhs=qT_sb[:, dt, :],
                    start=(dt == 0), stop=(dt == 1),
                )
            nc.scalar.copy(qf_sb[:, h2, :], ps[:, :])

        for b in seq[1:]:
            if b not in kf_tiles:
                kf_tiles[b] = project_kf(b, f"p{b}")

        for bi, b in enumerate(seq):
            Ku = ku[b]
            nkt = (Ku + 127) // 128
            kf_sb = kf_tiles[b]
            n_poly, n_pool = plan[b]
            na = QL - n_poly

            # scoresT[k, q] accumulated in one PSUM bank: [128, (kt, q)]
            sc_ps = ps_sc.tile([128, nkt, QL], F32, tag="sc")

            is_first = bi == 0
            is_last = bi == len(seq) - 1

            def ramp_up(n):
                gs = [2, 6, 8] if is_first else []
                done = sum(gs)
                while done < n:
                    g_ = min(GS, n - done)
                    gs.append(g_)
                    done += g_
                return [g for g in gs if g > 0]

            groups = ramp_up(na)
            # per-group Pool share: spread n_pool evenly over groups
            pool_left = n_pool
            # poly chunks interleave between ACT groups (after group 1) so
            # DVE's stream stays: [g0 pre, g1 pre, chunk, g2 pre, chunk, ...]
            chunks = []
            for c0_ in range(0, n_poly, PCH):
                chunks.append((c0_, min(PCH, n_poly - c0_)))

            def emit_chunk(c0_, cnt):
                pred = pred_pool.tile([128, PCH, 2, mku], F16, tag="pred")
                for j in range(cnt):
                    col = b * QL + na + c0_ + j
                    for h2 in range(2):
                        nc.vector.tensor_scalar(
                            pred[:, j, h2, :Ku],
                            kf_sb[:, h2, :Ku],
                            qf_sb[:, h2, col:col + 1], XC,
                            op0=OP.add, op1=OP.min,
                        )
                xcv = pred[:, :cnt, :, :Ku].rearrange("p a b c -> p (a b) c")
                nc.vector.tensor_scalar(xcv, xcv, -XC, None, op0=OP.max)
                res = res_pool.tile([128, PCH, 2, mku], F16, tag="res")
                nc.vector._custom_dve(
                    TANH7,
                    out=res[:, :cnt, :, :Ku].rearrange("p a b c -> p (a b) c"),
                    in0=xcv, in1=c3_sb[:, :cnt * 2 * Ku],
                    s0=float(T7[2]), s1=float(T7[1]), imm2=float(T7[0]))
                for kt in range(nkt):
                    cs = min(128, Ku - kt * 128)
                    for j in range(cnt):
                        qq = na + c0_ + j
                        for h2 in range(2):
                            nc.tensor.matmul(
                                sc_ps[:cs, kt, qq:qq + 1],
                                lhsT=res[:, j, h2, kt * 128:kt * 128 + cs],
                                rhs=wv16_sb[:, h2:h2 + 1],
                                start=(h2 == 0), stop=(h2 == 1),
                            )

            q0 = 0
            for g, gs_ in enumerate(groups):
                pre = pre_pool.tile([128, GS, 2, mku], BF16, tag="pre")
                # Pool takes the tail q's of the group (DVE fills the head
                # fast so ACT can start; Pool had a head start from emission)
                gpool = min(gs_ // 2 if gs_ >= 4 else 0, pool_left)
                pool_left -= gpool
                for j in range(gs_):
                    col = b * QL + q0 + j
                    eng = nc.gpsimd if j >= gs_ - gpool else nc.vector
                    for h2 in range(2):
                        eng.tensor_scalar_add(
                            pre[:, j, h2, :Ku],
                            kf_sb[:, h2, :Ku],
                            qf_sb[:, h2, col:col + 1],
                        )
                nc.scalar.activation(pre[:, :gs_, :, :Ku],
                                     pre[:, :gs_, :, :Ku], AF.Tanh)
                # kt-outer: the last group completes k-tile 0's scores first,
                # so exp/output-matmul overlap the remaining matvecs
                for kt in range(nkt):
                    cs = min(128, Ku - kt * 128)
                    for j in range(gs_):
                        qq = q0 + j
                        for h2 in range(2):
                            nc.tensor.matmul(
                                sc_ps[:cs, kt, qq:qq + 1],
                                lhsT=pre[:, j, h2, kt * 128:kt * 128 + cs],
                                rhs=wv_sb[:, h2:h2 + 1],
                                start=(h2 == 0), stop=(h2 == 1),
                            )
                q0 += gs_
                if g >= 1 and chunks:
                    emit_chunk(*chunks.pop(0))
            while chunks:
                emit_chunk(*chunks.pop(0))

            # values are only needed for the epilogue matmul; DMA them late
            vals_sb = sb.tile([128, 4, DA], BF16, tag="vals")
            nc.sync.dma_start(vals_sb[:, :nkt, :], vals[b, :, :nkt, :])

            # exp (mask = per-partition bias on the last k-tile), attn @ values
            cs_l = Ku - (nkt - 1) * 128
            masked = valid[b] < Ku
            e_all = exp_pool.tile([128, nkt, QL], BF16, tag="exp")
            if nkt > 1:
                nc.scalar.activation(e_all[:, :nkt - 1, :],
                                     sc_ps[:, :nkt - 1, :], AF.Exp)
            nc.scalar.activation(
                e_all[:cs_l, nkt - 1, :], sc_ps[:cs_l, nkt - 1, :], AF.Exp,
                bias=mb_sb[:cs_l, b:b + 1] if masked else 0.0,
            )
            out_ps = ps_out.tile([QL, DA], F32, tag="out")
            for kt in range(nkt):
                cs = min(128, Ku - kt * 128)
                nc.tensor.matmul(
                    out_ps[:, :],
                    lhsT=e_all[:cs, kt, :],
                    rhs=vals_sb[:cs, kt, :],
                    start=(kt == 0), stop=(kt == nkt - 1),
                )

            rcp = small.tile([QL, 1], F32, tag="rcp")
            nc.vector.reciprocal(rcp[:, :], out_ps[:, D:DA])
            out_sb = sb.tile([QL, D], F32, tag="osb")
            nc.vector.tensor_scalar_mul(out_sb[:, :], out_ps[:, :D], rcp[:, 0:1])
            nc.sync.dma_start(out[b], out_sb[:, :])

    nc.compile()
    return nc


def prep_inputs(queries, keys, values, Wq, Wk, wv, valid_lens):
    """Host-side layout prep (transposes/casts only). Returns per-core in_maps."""
    bf = ml_dtypes.bfloat16
    valid = [int(v) for v in valid_lens]
    kT = np.ascontiguousarray(
        keys.transpose(0, 2, 1).reshape(B, 2, 128, K).transpose(0, 2, 1, 3)
    ).astype(bf)  # [B, 128, 2, K]
    va = np.ones((B, K, DA), dtype=np.float32)
    va[:, :, :D] = values
    vals = np.ascontiguousarray(
        va.reshape(B, 4, 128, DA).transpose(0, 2, 1, 3)
    ).astype(bf)  # [B, 128, 4, DA]
    wq = np.ascontiguousarray(
        Wq.reshape(2, 128, H).transpose(1, 0, 2)).astype(bf)  # [128, 2, H]
    wk = np.ascontiguousarray(
        Wk.reshape(2, 128, H).transpose(1, 0, 2)).astype(bf)
    wv2 = np.ascontiguousarray(wv.reshape(2, 128).T).astype(bf)
    wv16 = np.ascontiguousarray(wv.reshape(2, 128).T).astype(np.float16)
    c3rep_h = np.full((128, PCH * 2 * K), T7[3], dtype=np.float16)
    mb = np.zeros((128, B), dtype=np.float32)
    for b in range(B):
        lastk0 = ((_k_use(valid[b]) + 127) // 128 - 1) * 128
        mb[:, b] = np.where(lastk0 + np.arange(128) < valid[b], 0.0, -40.0)
    in_maps = []
    for c in range(NCORES):
        qs = queries[:, c * QL:(c + 1) * QL, :]  # [B, QL, D]
        qTc = np.ascontiguousarray(
            qs.transpose(2, 0, 1).reshape(2, 128, B * QL).transpose(1, 0, 2)
        ).astype(bf)  # [128, 2, B*QL]
        in_maps.append({
            "qT": qTc, "kT": kT, "vals": vals,
            "wq": wq, "wk": wk, "wv2": wv2, "wv16": wv16, "mb": mb,
            "c3rep": c3rep_h,
        })
    return in_maps


def kernel(queries, keys, values, Wq, Wk, wv, valid_lens) -> np.ndarray:
    global last_results
    queries = np.asarray(queries, dtype=np.float32)
    keys = np.asarray(keys, dtype=np.float32)
    values = np.asarray(values, dtype=np.float32)
    Wq = np.asarray(Wq, dtype=np.float32)
    Wk = np.asarray(Wk, dtype=np.float32)
    wv = np.asarray(wv, dtype=np.float32)
    valid = tuple(int(v) for v in np.asarray(valid_lens))

    if valid not in _build_cache:
        _build_cache[valid] = build(valid)
    nc = _build_cache[valid]

    in_maps = prep_inputs(queries, keys, values, Wq, Wk, wv, valid)
    try:
        res = run_bass_kernel_spmd(nc, in_maps, core_ids=list(range(NCORES)))
    except Exception:
        # transient NRT device errors (wedged core) usually clear on retry
        res = run_bass_kernel_spmd(nc, in_maps, core_ids=list(range(NCORES)))
    last_results = res

    full = np.empty((B, Q, D), dtype=np.float32)
    for c in range(NCORES):
        oc = res.results[c]["out"]  # [B, QL, D]
        for b in range(B):
            full[b, c * QL:(c + 1) * QL, :] = oc[b]
    return full


# revision 13
# speedup vs baseline: 1.1753x; 1.1753x over previous
"""Additive attention (B=4, Q=512, K=512, D=256, H=256) on 8 TRN2 NeuronCores.

Sharding: data-parallel over query rows. Core c owns q-rows [c*64, (c+1)*64)
of every batch; every core holds all keys/values/weights. No collectives.

Per-core pipeline (per batch b, JIT-specialized to valid_lens[b]):
  PE : qf = Wq^T q, kf = Wk^T k (all batches upfront; bf16, H on partitions)
  then each q-row takes one of two tanh paths, balanced across engines:
   ACT path: pre[h,k] = kf[h,k] + qf[h,q] (DVE or Pool tensor_scalar),
             feat = tanh(pre) on ACT (batched groups of 16 q's)
   DVE path: xc = clamp(kf + qf, +-3.0) (two tensor_scalar), then a single
             8-stage custom-DVE op TANH7_ANT evaluates an odd deg-7 minimax
             polynomial: feat = xc*(c0 + c1 t + c2 t^2 + c3 t^3), t = xc^2
  PE : scoresT[k,q] = (feat as weights)^T wv  (k on PSUM partitions)
  ACT: expT[k,q] = exp(scoresT)  (masked tail rows biased to -40)
  PE : out[q,:] = expT^T @ [values | 1]  (ones col gives the softmax denom)
  DVE: out[:, :D] *= 1/out[:, D]
"""

from contextlib import ExitStack

import ml_dtypes
import numpy as np

import concourse.bass as bass
import concourse.mybir as mybir
import concourse.tile as tile
from concourse import bacc
from concourse.bass_utils import run_bass_kernel_spmd

B, Q, K, D, H = 4, 512, 512, 256, 256
DA = D + 1  # values with an appended ones column
NCORES = 8
QL = Q // NCORES  # 64 q-rows per core
GS = 8  # q's per ACT tanh group
PCH = 6  # q's per DVE poly chunk
BF16 = mybir.dt.bfloat16
F16 = mybir.dt.float16
F32 = mybir.dt.float32
AF = mybir.ActivationFunctionType
OP = mybir.AluOpType

# Custom-DVE tanh: odd deg-7 minimax on [-XC, XC], one 8-stage instruction
# (tanh(x) ~ x*(c0 + c1 t + c2 t^2 + c3 t^3), t = x^2; |err| < 2.2e-2 + clamp
# err 5e-3 -- end-to-end rel err ~7e-3 with half the q-rows on this path).
XC = 3.0
T7 = [0.95467697, -0.21116664, 0.02812073, -0.00138176]  # c0..c3

_build_cache: dict = {}
last_results = None  # BassKernelResults of the most recent kernel() call


def _register_tanh7():
    import concourse.dve_ops as dops
    from concourse.dve_ops import DveOp
    from concourse.dve_spec import Spec, Src0, Src1, C0, C1, C2, sq, lower
    from concourse.dve_uop import DveOpSpec

    if "TANH7_ANT" in dops._SUB_OPCODE_FOR_NAME:
        return next(op for op in dops.OPS if op.name == "TANH7_ANT")

    def _tanh7_ref(in0, in1, c0, c1, c2):
        x = in0.astype(np.float32).reshape(in0.shape[0], -1)
        if isinstance(in1, np.ndarray):
            in1 = np.asarray(in1, np.float32).reshape(in1.shape[0], -1)
            if in1.shape[1] not in (1, x.shape[1]):
                in1 = in1[:, :1]
        t = x * x
        u = in1 * t + c0
        u = u * t + c1
        u = u * t + c2
        return u * x

    _t = sq(Src0)
    _u = Src1 * _t + C0
    _u = _u * _t + C1
    _u = _u * _t + C2
    op = DveOp("TANH7_ANT", Spec(body=_u * Src0, reference=_tanh7_ref),
               subdim=False, uops_sha={})
    row = max(dops._SUB_OPCODE_FOR_NAME.values()) + 1
    dops.OPS.append(op)
    dops._SUB_OPCODE_FOR_NAME[op.name] = row
    dops.CUSTOM_DVE_SPECS[op.name] = op.spec
    for ver in ("v3",):
        s = DveOpSpec(name=op.name, opcode=row, uops=lower(op.spec, ver=ver),
                      rd1_en=True)
        op.uops_sha[ver] = s.sha(ver)
    return op


TANH7 = _register_tanh7()


def _k_use(v: int) -> int:
    # columns actually computed for a batch: valid len rounded up to a
    # multiple of 4 (even FD keeps DVE tensor_scalar in fast mode)
    return min(K, max(4, ((int(v) + 3) // 4) * 4))


def _plan(ku):
    """Per-batch (n_poly, n_pool): q-rows on the DVE-poly path and q-rows
    whose ACT-path preadd runs on Pool/gpsimd. Greedy 2-D grid on the
    analytic per-engine cost model (ns)."""
    def tot(fp, fl):
        act = dve = pool = 0.0
        for Ku in ku:
            np_ = int(round(fp * QL))
            nl_ = int(round(fl * QL))
            if np_ + nl_ > QL:
                nl_ = QL - np_
            na = QL - np_
            nd = max(na - nl_, 0)
            act += na * 2 * Ku * 0.8333
            dve += nd * 2 * (0.2604 * Ku + 60) \
                + np_ * 2 * (0.2604 * Ku + 60) \
                + np_ * 2 * Ku * (0.2604 + 1.0417) + 120
            pool += nl_ * 2 * (1.3889 * Ku + 156)
        return max(act, dve, pool)

    best = (1e18, 0.0, 0.0)
    for fp10 in range(0, 40):
        for fl10 in range(0, 55):
            v = tot(fp10 / 64.0, fl10 / 64.0)
            if v < best[0]:
                best = (v, fp10 / 64.0, fl10 / 64.0)
    _, fp, fl = best
    out = []
    for Ku in ku:
        np_ = int(round(fp * QL))
        nl_ = min(int(round(fl * QL)), QL - np_)
        nl_ = (nl_ // GS) * GS  # Pool owns whole groups
        out.append((np_, nl_))
    return out


def build(valid_lens, repeat: int = 1) -> bacc.Bacc:
    valid = tuple(int(v) for v in valid_lens)
    ku = [_k_use(v) for v in valid]

    nc = bacc.Bacc("TRN2", target_bir_lowering=False, debug=False,
                   enable_asserts=False)

    qT = nc.dram_tensor("qT", [128, 2, B * QL], BF16, kind="ExternalInput").ap()
    kT = nc.dram_tensor("kT", [B, 128, 2, K], BF16, kind="ExternalInput").ap()
    vals = nc.dram_tensor("vals", [B, 128, 4, DA], BF16, kind="ExternalInput").ap()
    wq = nc.dram_tensor("wq", [128, 2, H], BF16, kind="ExternalInput").ap()
    wk = nc.dram_tensor("wk", [128, 2, H], BF16, kind="ExternalInput").ap()
    wv2 = nc.dram_tensor("wv2", [128, 2], BF16, kind="ExternalInput").ap()
    wv16 = nc.dram_tensor("wv16", [128, 2], F16, kind="ExternalInput").ap()
    # per-batch mask bias column for the last k-tile's exp: 0 on valid rows,
    # -40 on the rounded-up tail (exp(-40) ~ 4e-18 ~ 0)
    mb = nc.dram_tensor("mb", [128, B], F32, kind="ExternalInput").ap()
    # constant c3 plane for the custom op's second stream ([P,1] broadcast
    # src1 crashes this firmware; a real full-size tensor works)
    c3rep = nc.dram_tensor("c3rep", [128, PCH * 2 * K], F16,
                           kind="ExternalInput").ap()
    out = nc.dram_tensor("out", [B, QL, D], F32, kind="ExternalOutput").ap()

    plan = _plan(ku)
    mku = max(ku)

    with tile.TileContext(nc) as tc, ExitStack() as ctx:
        cp = ctx.enter_context(tc.tile_pool(name="consts", bufs=1))
        sb = ctx.enter_context(tc.tile_pool(name="sbuf", bufs=2))
        exp_pool = ctx.enter_context(tc.tile_pool(name="expp", bufs=5))
        small = ctx.enter_context(tc.tile_pool(name="small", bufs=4))
        pre_pool = ctx.enter_context(tc.tile_pool(name="pre", bufs=3))
        pool_pre = ctx.enter_context(tc.tile_pool(name="lpre", bufs=2))
        pred_pool = ctx.enter_context(tc.tile_pool(name="pred", bufs=2))
        res_pool = ctx.enter_context(tc.tile_pool(name="res", bufs=2))
        ps_proj = ctx.enter_context(tc.tile_pool(name="ps_proj", bufs=2, space="PSUM"))
        ps_sc = ctx.enter_context(tc.tile_pool(name="ps_sc", bufs=3, space="PSUM"))
        ps_out = ctx.enter_context(tc.tile_pool(name="ps_out", bufs=2, space="PSUM"))

        # prime the ACT table load (tanh/exp share one set) at t~0 so the
        # ~2.7us load is off the critical path
        primer = cp.tile([1, 1], F32, tag="primer")
        nc.gpsimd.memset(primer[:, :], 0.0)
        nc.scalar.activation(primer[:, :], primer[:, :], AF.Tanh)

        wq_sb = cp.tile([128, 2, H], BF16, tag="wq")
        wk_sb = cp.tile([128, 2, H], BF16, tag="wk")
        wv_sb = cp.tile([128, 2], BF16, tag="wv")
        wv16_sb = cp.tile([128, 2], F16, tag="wv16")
        qT_sb = cp.tile([128, 2, B * QL], BF16, tag="qT")
        mb_sb = cp.tile([128, B], F32, tag="mb")
        c3_sb = cp.tile([128, PCH * 2 * K], F16, tag="c3")

        # process batches small-to-large so the serial prologue (DMA ->
        # projection -> preadd -> first tanh) is as short as possible
        border = sorted(range(B), key=lambda b: ku[b])
        seq = [b for _ in range(repeat) for b in border]
        b0 = seq[0]

        # spread the prologue-critical DMAs over different engines' DMA
        # queues so they transfer in parallel
        kT_sb = cp.tile([128, B, 2, K], BF16, tag="kT")
        nc.scalar.dma_start(kT_sb[:, b0], kT[b0])
        nc.sync.dma_start(wk_sb[:, :, :], wk[:, :, :])
        nc.sync.dma_start(wq_sb[:, :, :], wq[:, :, :])
        nc.gpsimd.dma_start(qT_sb[:, :, :], qT[:, :, :])
        nc.sync.dma_start(wv_sb[:, :], wv2[:, :])
        nc.sync.dma_start(wv16_sb[:, :], wv16[:, :])
        nc.sync.dma_start(mb_sb[:, :], mb[:, :])
        nc.scalar.dma_start(c3_sb[:, :], c3rep[:, :])
        for b in seq[1:]:
            nc.sync.dma_start(kT_sb[:, b], kT[b])

        # kf projections for every batch upfront (PE is far from saturated);
        # kf[h, k] bf16, H split in two 128-halves
        kf_tiles = {}

        def project_kf(b, uniq):
            Ku_ = ku[b]
            kf_t = cp.tile([128, 2, K], BF16, tag=f"kf{b}", name=f"kf_{uniq}")
            for h2 in range(2):
                ps = ps_proj.tile([128, K], F32, tag="proj",
                                  name=f"kfp_{uniq}_{h2}")
                for dt in range(2):
                    nc.tensor.matmul(
                        ps[:, :Ku_],
                        lhsT=wk_sb[:, dt, bass.ts(h2, 128)],
                        rhs=kT_sb[:, b, dt, :Ku_],
                        start=(dt == 0), stop=(dt == 1),
                    )
                nc.vector.tensor_copy(kf_t[:, h2, :Ku_], ps[:, :Ku_])
            return kf_t

        kf_tiles[b0] = project_kf(b0, "p0")

        # qf[h, (b,q)] for all batches, f32 (tensor_scalar's per-partition
        # scalar operand must be float32; copies go on the ACT engine)
        qf_sb = cp.tile([128, 2, B * QL], F32, tag="qf")
        for h2 in range(2):
            ps = ps_proj.tile([128, B * QL], F32, tag="proj",
                              name=f"qfp_{h2}")
            for dt in range(2):
                nc.tensor.matmul(
                    ps[:, :],
                    lhsT=wq_sb[:, dt, bass.ts(h2, 128)],
                    rhs=qT_sb[:, dt, :],
                    start=(dt == 0), stop=(dt == 1),
                )
            nc.scalar.copy(qf_sb[:, h2, :], ps[:, :])

        for b in seq[1:]:
            if b not in kf_tiles:
                kf_tiles[b] = project_kf(b, f"p{b}")

        for bi, b in enumerate(seq):
            Ku = ku[b]
            nkt = (Ku + 127) // 128
            kf_sb = kf_tiles[b]
            n_poly, n_pool = plan[b]
            na = QL - n_poly

            # scoresT[k, q] accumulated in one PSUM bank: [128, (kt, q)]
            sc_ps = ps_sc.tile([128, nkt, QL], F32, tag="sc")

            is_first = bi == 0
            is_last = bi == len(seq) - 1

            def ramp_up(n):
                gs = [2, 6, 8] if is_first else []
                done = sum(gs)
                while done < n:
                    g_ = min(GS, n - done)
                    gs.append(g_)
                    done += g_
                return [g for g in gs if g > 0]

            groups = ramp_up(na - n_pool)
            # poly chunks interleave between ACT groups (after group 1) so
            # DVE's stream stays: [g0 pre, g1 pre, chunk, g2 pre, chunk, ...]
            chunks = []
            for c0_ in range(0, n_poly, PCH):
                chunks.append((c0_, min(PCH, n_poly - c0_)))

            def emit_chunk(c0_, cnt):
                pred = pred_pool.tile([128, PCH, 2, mku], F16, tag="pred")
                for j in range(cnt):
                    col = b * QL + na + c0_ + j
                    for h2 in range(2):
                        nc.vector.tensor_scalar(
                            pred[:, j, h2, :Ku],
                            kf_sb[:, h2, :Ku],
                            qf_sb[:, h2, col:col + 1], XC,
                            op0=OP.add, op1=OP.min,
                        )
                xcv = pred[:, :cnt, :, :Ku].rearrange("p a b c -> p (a b) c")
                nc.vector.tensor_scalar(xcv, xcv, -XC, None, op0=OP.max)
                res = res_pool.tile([128, PCH, 2, mku], F16, tag="res")
                nc.vector._custom_dve(
                    TANH7,
                    out=res[:, :cnt, :, :Ku].rearrange("p a b c -> p (a b) c"),
                    in0=xcv, in1=c3_sb[:, :cnt * 2 * Ku],
                    s0=float(T7[2]), s1=float(T7[1]), imm2=float(T7[0]))
                for kt in range(nkt):
                    cs = min(128, Ku - kt * 128)
                    for j in range(cnt):
                        qq = na + c0_ + j
                        for h2 in range(2):
                            nc.tensor.matmul(
                                sc_ps[:cs, kt, qq:qq + 1],
                                lhsT=res[:, j, h2, kt * 128:kt * 128 + cs],
                                rhs=wv16_sb[:, h2:h2 + 1],
                                start=(h2 == 0), stop=(h2 == 1),
                            )

            # Pool owns the last n_pool ACT q's as whole groups: its
            # preadds are emitted first (Pool starts at batch begin) and
            # their tanh groups run last on ACT, so Pool's latency hides
            # behind the DVE-fed groups.
            n_dve_act = na - n_pool

            def matvec(src_tile, j, qq, f16):
                for kt in range(nkt):
                    cs = min(128, Ku - kt * 128)
                    for h2 in range(2):
                        nc.tensor.matmul(
                            sc_ps[:cs, kt, qq:qq + 1],
                            lhsT=src_tile[:, j, h2, kt * 128:kt * 128 + cs],
                            rhs=(wv16_sb if f16 else wv_sb)[:, h2:h2 + 1],
                            start=(h2 == 0), stop=(h2 == 1),
                        )

            pool_tiles = []
            for pg in range(n_pool // GS):
                qbase = n_dve_act + pg * GS
                pp = pool_pre.tile([128, GS, 2, mku], BF16, tag="lpre")
                for j in range(GS):
                    col = b * QL + qbase + j
                    for h2 in range(2):
                        nc.gpsimd.tensor_scalar_add(
                            pp[:, j, h2, :Ku],
                            kf_sb[:, h2, :Ku],
                            qf_sb[:, h2, col:col + 1],
                        )
                pool_tiles.append((pp, qbase))

            q0 = 0
            for g, gs_ in enumerate(groups):
                pre = pre_pool.tile([128, GS, 2, mku], BF16, tag="pre")
                for j in range(gs_):
                    col = b * QL + q0 + j
                    for h2 in range(2):
                        nc.vector.tensor_scalar_add(
                            pre[:, j, h2, :Ku],
                            kf_sb[:, h2, :Ku],
                            qf_sb[:, h2, col:col + 1],
                        )
                nc.scalar.activation(pre[:, :gs_, :, :Ku],
                                     pre[:, :gs_, :, :Ku], AF.Tanh)
                # kt-outer: the last group completes k-tile 0's scores first,
                # so exp/output-matmul overlap the remaining matvecs
                for j in range(gs_):
                    matvec(pre, j, q0 + j, False)
                q0 += gs_
                if g >= 1 and chunks:
                    emit_chunk(*chunks.pop(0))
            while chunks:
                emit_chunk(*chunks.pop(0))
            for pp, qbase in pool_tiles:
                nc.scalar.activation(pp[:, :, :, :Ku], pp[:, :, :, :Ku],
                                     AF.Tanh)
                for j in range(GS):
                    matvec(pp, j, qbase + j, False)

            # values are only needed for the epilogue matmul; DMA them late
            vals_sb = sb.tile([128, 4, DA], BF16, tag="vals")
            nc.sync.dma_start(vals_sb[:, :nkt, :], vals[b, :, :nkt, :])

            # exp (mask = per-partition bias on the last k-tile), attn @ values
            cs_l = Ku - (nkt - 1) * 128
            masked = valid[b] < Ku
            e_all = exp_pool.tile([128, nkt, QL], BF16, tag="exp")
            if nkt > 1:
                nc.scalar.activation(e_all[:, :nkt - 1, :],
                                     sc_ps[:, :nkt - 1, :], AF.Exp)
            nc.scalar.activation(
                e_all[:cs_l, nkt - 1, :], sc_ps[:cs_l, nkt - 1, :], AF.Exp,
                bias=mb_sb[:cs_l, b:b + 1] if masked else 0.0,
            )
            out_ps = ps_out.tile([QL, DA], F32, tag="out")
            for kt in range(nkt):
                cs = min(128, Ku - kt * 128)
                nc.tensor.matmul(
                    out_ps[:, :],
                    lhsT=e_all[:cs, kt, :],
                    rhs=vals_sb[:cs, kt, :],
                    start=(kt == 0), stop=(kt == nkt - 1),
                )

            rcp = small.tile([QL, 1], F32, tag="rcp")
            nc.vector.reciprocal(rcp[:, :], out_ps[:, D:DA])
            out_sb = sb.tile([QL, D], F32, tag="osb")
            nc.vector.tensor_scalar_mul(out_sb[:, :], out_ps[:, :D], rcp[:, 0:1])
            nc.sync.dma_start(out[b], out_sb[:, :])

    nc.compile()
    return nc


def prep_inputs(queries, keys, values, Wq, Wk, wv, valid_lens):
    """Host-side layout prep (transposes/casts only). Returns per-core in_maps."""
    bf = ml_dtypes.bfloat16
    valid = [int(v) for v in valid_lens]
    kT = np.ascontiguousarray(
        keys.transpose(0, 2, 1).reshape(B, 2, 128, K).transpose(0, 2, 1, 3)
    ).astype(bf)  # [B, 128, 2, K]
    va = np.ones((B, K, DA), dtype=np.float32)
    va[:, :, :D] = values
    vals = np.ascontiguousarray(
        va.reshape(B, 4, 128, DA).transpose(0, 2, 1, 3)
    ).astype(bf)  # [B, 128, 4, DA]
    wq = np.ascontiguousarray(
        Wq.reshape(2, 128, H).transpose(1, 0, 2)).astype(bf)  # [128, 2, H]
    wk = np.ascontiguousarray(
        Wk.reshape(2, 128, H).transpose(1, 0, 2)).astype(bf)
    wv2 = np.ascontiguousarray(wv.reshape(2, 128).T).astype(bf)
    wv16 = np.ascontiguousarray(wv.reshape(2, 128).T).astype(np.float16)
    c3rep_h = np.full((128, PCH * 2 * K), T7[3], dtype=np.float16)
    mb = np.zeros((128, B), dtype=np.float32)
    for b in range(B):
        lastk0 = ((_k_use(valid[b]) + 127) // 128 - 1) * 128
        mb[:, b] = np.where(lastk0 + np.arange(128) < valid[b], 0.0, -40.0)
    in_maps = []
    for c in range(NCORES):
        qs = queries[:, c * QL:(c + 1) * QL, :]  # [B, QL, D]
        qTc = np.ascontiguousarray(
            qs.transpose(2, 0, 1).reshape(2, 128, B * QL).transpose(1, 0, 2)
        ).astype(bf)  # [128, 2, B*QL]
        in_maps.append({
            "qT": qTc, "kT": kT, "vals": vals,
            "wq": wq, "wk": wk, "wv2": wv2, "wv16": wv16, "mb": mb,
            "c3rep": c3rep_h,
        })
    return in_maps


def kernel(queries, keys, values, Wq, Wk, wv, valid_lens) -> np.ndarray:
    global last_results
    queries = np.asarray(queries, dtype=np.float32)
    keys = np.asarray(keys, dtype=np.float32)
    values = np.asarray(values, dtype=np.float32)
    Wq = np.asarray(Wq, dtype=np.float32)
    Wk = np.asarray(Wk, dtype=np.float32)
    wv = np.asarray(wv, dtype=np.float32)
    valid = tuple(int(v) for v in np.asarray(valid_lens))

    if valid not in _build_cache:
        _build_cache[valid] = build(valid)
    nc = _build_cache[valid]

    in_maps = prep_inputs(queries, keys, values, Wq, Wk, wv, valid)
    try:
        res = run_bass_kernel_spmd(nc, in_maps, core_ids=list(range(NCORES)))
    except Exception:
        # transient NRT device errors (wedged core) usually clear on retry
        res = run_bass_kernel_spmd(nc, in_maps, core_ids=list(range(NCORES)))
    last_results = res

    full = np.empty((B, Q, D), dtype=np.float32)
    for c in range(NCORES):
        oc = res.results[c]["out"]  # [B, QL, D]
        for b in range(B):
            full[b, c * QL:(c + 1) * QL, :] = oc[b]
    return full
